# revision 1
# baseline (speedup 1.0000x reference)
"""CFConvS2V Trainium2 kernel (8-core data-parallel over batch).

reference computation:
    h = silu(layernorm(s @ W1.T + b1))               # (B, N, H)
    v[b,i,c,d] = sum_j mask[b,i,j] * ev[b,i,j,c] * h[b,j,d]   # (B, N, 3, H)

Sharding: data-parallel over B across 8 cores (4 batches each); the pairwise
tensors and the j-reduction stay local per core.

Per-core plan (B_loc=4, N=512, H=128, C=3):
  h-phase (per batch): 4 fp32 matmuls (sT chunk stationary, W1T moving) into
  PSUM, +b1, LayerNorm via bn_stats/bn_aggr, then Sigmoid*x for SiLU.
  i-tile phase (per 128 rows of i): one DVE multiply applies the mask to ev
  in natural [i,(j,c)] layout (mask broadcast over c via a 0-stride AP --
  keeps all mask work off the PE, whose transpose-mode ops never warm the
  HAM clock gate and so run at 1.2 GHz); PE transposes the 12 masked ev
  blocks into PSUM (f32r transpose mode, 1.5 cyc/row, value-exact); ACT
  evicts them to SBUF; 4 accumulating float32r matmuls with h[jc] stationary
  and mevT [j,(c,i)] moving (N=384 >= 256 keeps f32r at full 1 cyc/row);
  ACT evicts v [d,(c,i)]; stores ride the ACT HWDGE ring so they can't block
  the SP-ring loads. Host reorders [d,(c,i)] -> [i,c,d].
"""

import sys

if "/opt/trn_rl_repo" not in sys.path:
    sys.path.insert(0, "/opt/trn_rl_repo")

from contextlib import ExitStack

import numpy as np

import concourse.bass as bass
import concourse.mybir as mybir
from concourse.tile import TileContext

B, N, H, C = 32, 512, 128, 3
NCORES = 8
BL = B // NCORES      # batches per core
P = 128
NT = N // P           # i-tiles per batch
JC = N // P           # j-chunks
LN_EPS = 1e-5
F32 = mybir.dt.float32
F32R = mybir.dt.float32r
AF = mybir.ActivationFunctionType

# compute dtype for the big contraction: "f32r" (full-rate, reduced-precision
# single-pass fp32) or "f32" (exact, 4 cycles/row)
MM_DTYPE = "f32r"


def _split_multi_waits(nc):
    """The walrus build in this container only accepts one sync-wait per
    instruction; hoist extra waits onto single-wait NOPs in front."""
    ctr = 0
    for f in nc.m.functions:
        for bb in f.blocks:
            insts = bb.instructions
            i = 0
            while i < len(insts):
                inst = insts[i]
                si = inst.sync_info
                if si is not None and len(si.on_wait) > 1:
                    waits = list(si.on_wait)
                    for w in waits[:-1]:
                        ctr += 1
                        nop = mybir.InstNoOp(
                            name=f"splitwait-{ctr}",
                            engine=inst.engine,
                            sync_info=mybir.SyncInfo(on_wait=[w], on_update=[]),
                            bass_nofuse=True,
                        )
                        nc.register_instruction(nop, overwrite=True)
                        insts.insert(i, nop)
                        i += 1
                    inst.sync_info = mybir.SyncInfo(
                        on_wait=[waits[-1]], on_update=list(si.on_update)
                    )
                i += 1


def build(reps=1):
    nc = bass.Bass("TRN2", target_bir_lowering=False, debug=False, num_devices=NCORES)
    ev = nc.dram_tensor("ev", [BL, N, N * C], F32R, kind="ExternalInput").ap()
    mask = nc.dram_tensor("mask", [BL, N, N], F32R, kind="ExternalInput").ap()
    sT = nc.dram_tensor("sT", [BL, H, N], F32, kind="ExternalInput").ap()
    w1t = nc.dram_tensor("w1t", [H, H], F32, kind="ExternalInput").ap()
    b1b = nc.dram_tensor("b1b", [P, H], F32, kind="ExternalInput").ap()
    out = nc.dram_tensor("out", [BL, NT, H, C * P], F32, kind="ExternalOutput").ap()

    with TileContext(nc) as tc, ExitStack() as ctx:
        const = ctx.enter_context(tc.tile_pool(name="const", bufs=1))
        p_ev = ctx.enter_context(tc.tile_pool(name="p_ev", bufs=3))
        p_mask = ctx.enter_context(tc.tile_pool(name="p_mask", bufs=3))
        p_mev = ctx.enter_context(tc.tile_pool(name="p_mev", bufs=3))
        p_mevT = ctx.enter_context(tc.tile_pool(name="p_mevT", bufs=3))
        p_vout = ctx.enter_context(tc.tile_pool(name="p_vout", bufs=3))
        p_sT = ctx.enter_context(tc.tile_pool(name="p_sT", bufs=2))
        p_xb = ctx.enter_context(tc.tile_pool(name="p_xb", bufs=2))
        p_h = ctx.enter_context(tc.tile_pool(name="p_h", bufs=2))
        p_stat = ctx.enter_context(tc.tile_pool(name="p_stat", bufs=4))
        ps_eT = ctx.enter_context(tc.tile_pool(name="ps_eT", bufs=4, space="PSUM"))
        ps_v = ctx.enter_context(tc.tile_pool(name="ps_v", bufs=2, space="PSUM"))
        ps_h = ctx.enter_context(tc.tile_pool(name="ps_h", bufs=1, space="PSUM"))

        # identity in f32r, produced natively in f32r so the verifier sees a
        # rounded producer for the f32r transpose-matmuls
        ident = const.tile([P, P], F32R)
        nc.gpsimd.memset(ident[:].bitcast(F32), 0.0)
        nc.gpsimd.affine_select(
            out=ident[:], in_=ident[:],
            compare_op=mybir.AluOpType.not_equal, fill=1.0,
            base=0, pattern=[[-1, P]], channel_multiplier=1,
        )
        w1t_sb = const.tile([H, H], F32)
        nc.sync.dma_start(out=w1t_sb[:], in_=w1t[:])
        b1b_sb = const.tile([P, H], F32)
        nc.sync.dma_start(out=b1b_sb[:], in_=b1b[:])
        eps_sb = const.tile([P, 1], F32)
        nc.vector.memset(eps_sb[:], LN_EPS)

        def body():
          for b in range(BL):
            # ---------- h phase: h = silu(LN(s @ W1.T + b1)) ----------
            sT_sb = p_sT.tile([H, N], F32)
            nc.sync.dma_start(out=sT_sb[:], in_=sT[b])

            psum_h = ps_h.tile([P, N], F32)
            for t in range(NT):
                # out[n_local, k] = sum_h sT[h, n] * W1T[h, k]
                nc.tensor.matmul(
                    out=psum_h[:, t * P : (t + 1) * P],
                    lhsT=sT_sb[:, t * P : (t + 1) * P],
                    rhs=w1t_sb[:],
                    start=True,
                    stop=True,
                )

            xb = p_xb.tile([P, NT, H], F32)
            nc.vector.tensor_tensor(
                out=xb[:],
                in0=psum_h[:].rearrange("p (t k) -> p t k", k=H),
                in1=b1b_sb[:].unsqueeze(1).broadcast_to((P, NT, H)),
                op=mybir.AluOpType.add,
            )

            h_sb = p_h.tile([P, NT, H], F32R if MM_DTYPE == "f32r" else F32)
            for t in range(NT):
                stats = p_stat.tile([P, 6], F32)
                nc.vector.bn_stats(out=stats[:], in_=xb[:, t, :])
                mv = p_stat.tile([P, 2], F32)
                nc.vector.bn_aggr(out=mv[:], in_=stats[:])
                rstd = p_stat.tile([P, 1], F32, tag="rstd")
                nc.scalar.activation(
                    out=rstd[:], in_=mv[:, 1:2], func=AF.Sqrt, bias=eps_sb[:]
                )
                nc.vector.reciprocal(out=rstd[:], in_=rstd[:])
                # xn = (x - mu) * rstd; h = xn * sigmoid(xn)
                xn = p_stat.tile([P, H], F32, tag="xn")
                nc.vector.tensor_scalar(
                    out=xn[:],
                    in0=xb[:, t, :],
                    scalar1=mv[:, 0:1],
                    scalar2=rstd[:],
                    op0=mybir.AluOpType.subtract,
                    op1=mybir.AluOpType.mult,
                )
                sg = p_stat.tile([P, H], F32, tag="sg")
                nc.scalar.activation(out=sg[:], in_=xn[:], func=AF.Sigmoid)
                nc.vector.tensor_mul(out=h_sb[:, t, :], in0=xn[:], in1=sg[:])

            # ---------- i-tile phase ----------
            for it in range(NT):
                isl = slice(it * P, (it + 1) * P)
                ev_sb_t = p_ev.tile([P, N * C], F32R)
                nc.sync.dma_start(out=ev_sb_t[:], in_=ev[b, isl])
                ev_sb = ev_sb_t[:]
                mask_sb_t = p_mask.tile([P, N], F32R)
                nc.sync.dma_start(out=mask_sb_t[:], in_=mask[b, isl])
                mask_sb = mask_sb_t[:]

                # mev[i,(j,c)] = ev[i,(j,c)] * mask[i,j] (broadcast over c)
                # in natural layout: one DVE op, no mask transposes needed
                mev_sb = p_mev.tile([P, N * C], F32R)
                nc.vector.tensor_tensor(
                    out=mev_sb[:].rearrange("p (j c) -> p j c", c=C),
                    in0=ev_sb.rearrange("p (j c) -> p j c", c=C),
                    in1=mask_sb.unsqueeze(2).broadcast_to((P, N, C)),
                    op=mybir.AluOpType.mult,
                )

                psum_v = ps_v.tile([P, C * P], F32)
                ev3 = mev_sb[:].rearrange("p (j c) -> p j c", c=C)
                for jc in range(JC):
                    jsl = slice(jc * P, (jc + 1) * P)
                    psum_eT = ps_eT.tile([P, C * P], F32R)
                    for c in range(C):
                        nc.tensor.transpose(
                            out=psum_eT[:, c * P : (c + 1) * P],
                            in_=ev3[:, jsl, c],
                            identity=ident[:],
                        )
                    # plain eviction to SBUF on ACT (mask already applied)
                    mevT = p_mevT.tile([P, C * P], F32R if MM_DTYPE == "f32r" else F32)
                    nc.scalar.activation(out=mevT[:], in_=psum_eT[:], func=AF.Copy)
                    # v[d, (c,i)] += sum_j h[j, d] * mevT[j, (c,i)]
                    nc.tensor.matmul(
                        out=psum_v[:],
                        lhsT=h_sb[:, jc, :],
                        rhs=mevT[:],
                        start=(jc == 0),
                        stop=(jc == JC - 1),
                        skip_group_check=True,
                    )

                vout = p_vout.tile([P, C * P], F32)
                nc.scalar.activation(out=vout[:], in_=psum_v[:], func=AF.Copy)
                # store on the ACT HWDGE ring so a compute-gated store
                # can't block the next tile's loads on the SP HWDGE FIFO
                nc.scalar.dma_start(out=out[b, it], in_=vout[:])

        if reps == 1:
            body()
        else:
            with tc.For_i(0, reps, 1):
                body()

    _split_multi_waits(nc)
    return nc


_built_nc = None


def _get_nc():
    global _built_nc
    if _built_nc is None:
        _built_nc = build()
    return _built_nc


def shard_inputs(s, ev, mask, W1, b1):
    """Full inputs -> list of per-core input dicts."""
    s = np.asarray(s, dtype=np.float32)
    ev = np.asarray(ev, dtype=np.float32)
    mask = np.asarray(mask, dtype=np.float32)
    W1 = np.asarray(W1, dtype=np.float32)
    b1 = np.asarray(b1, dtype=np.float32)
    w1t = np.ascontiguousarray(W1.T)
    b1b = np.ascontiguousarray(np.broadcast_to(b1[None, :], (P, H)))
    in_maps = []
    for m in range(NCORES):
        bs = slice(m * BL, (m + 1) * BL)
        in_maps.append(
            {
                "ev": np.ascontiguousarray(ev[bs].reshape(BL, N, N * C)),
                "mask": np.ascontiguousarray(mask[bs].reshape(BL, N, N)),
                "sT": np.ascontiguousarray(s[bs].transpose(0, 2, 1)),
                "w1t": w1t,
                "b1b": b1b,
            }
        )
    return in_maps


def unshard_output(per_core_outs):
    """list of per-core "out" arrays [BL, NT, H, C*P] -> full (B, N, 3, H)."""
    parts = []
    for o in per_core_outs:
        o = o.reshape(BL, NT, H, C, P).transpose(0, 1, 4, 3, 2)
        parts.append(np.ascontiguousarray(o).reshape(BL, N, C, H))
    return np.concatenate(parts, axis=0)


_executor = None


def _get_executor():
    """Build the sharded PJRT executable once; reuse across kernel() calls."""
    global _executor
    if _executor is not None:
        return _executor
    import jax
    from jax.sharding import Mesh, PartitionSpec
    from jax.experimental.shard_map import shard_map

    from concourse import bass2jax

    bass2jax.install_neuronx_cc_hook()
    nc = _get_nc()
    partition_name = nc.partition_id_tensor.name if nc.partition_id_tensor else None
    in_names, out_names, out_avals, zero_outs = [], [], [], []
    for alloc in nc.m.functions[0].allocations:
        if not isinstance(alloc, mybir.MemoryLocationSet):
            continue
        name = alloc.memorylocations[0].name
        if alloc.kind == "ExternalInput":
            if name != partition_name:
                in_names.append(name)
        elif alloc.kind == "ExternalOutput":
            out_names.append(name)
            shape = tuple(alloc.tensor_shape)
            dtype = mybir.dt.np(alloc.dtype)
            out_avals.append(jax.core.ShapedArray(shape, dtype))
            zero_outs.append(np.zeros(shape, dtype))
    n_params = len(in_names)
    all_in_names = list(in_names) + list(out_names)
    if partition_name is not None:
        all_in_names.append(partition_name)

    def _body(*args):
        operands = list(args)
        if partition_name is not None:
            operands.append(bass2jax.partition_id_tensor())
        outs = bass2jax._bass_exec_p.bind(
            *operands,
            out_avals=tuple(out_avals),
            in_names=tuple(all_in_names),
            out_names=tuple(out_names),
            lowering_input_output_aliases=(),
            sim_require_finite=True,
            sim_require_nnan=True,
            nc=nc,
        )
        return tuple(outs)

    devices = jax.devices()[:NCORES]
    mesh = Mesh(np.asarray(devices), ("core",))
    donate = tuple(range(n_params, n_params + len(out_names)))
    fn = jax.jit(
        shard_map(
            _body,
            mesh=mesh,
            in_specs=(PartitionSpec("core"),) * (n_params + len(out_names)),
            out_specs=(PartitionSpec("core"),) * len(out_names),
            check_rep=False,
        ),
        donate_argnums=donate,
        keep_unused=True,
    )
    _executor = (fn, in_names, out_names, out_avals, zero_outs)
    return _executor


def kernel(s, ev, mask, W1, b1):
    fn, in_names, out_names, out_avals, zero_outs = _get_executor()
    in_maps = shard_inputs(s, ev, mask, W1, b1)
    concat_in = [
        np.concatenate([in_maps[c][nm] for c in range(NCORES)], axis=0)
        for nm in in_names
    ]
    concat_zeros = [
        np.zeros((NCORES * z.shape[0], *z.shape[1:]), z.dtype) for z in zero_outs
    ]
    out_arrs = fn(*concat_in, *concat_zeros)
    i = out_names.index("out")
    o = np.asarray(out_arrs[i]).reshape(NCORES, *out_avals[i].shape)
    return unshard_output([o[c] for c in range(NCORES)])



# revision 3
# speedup vs baseline: 1.0562x; 1.0562x over previous
"""CFConvS2V Trainium2 kernel (8-core data-parallel over batch).

reference computation:
    h = silu(layernorm(s @ W1.T + b1))               # (B, N, H)
    v[b,i,c,d] = sum_j mask[b,i,j] * ev[b,i,j,c] * h[b,j,d]   # (B, N, 3, H)

Sharding: data-parallel over B across 8 cores (4 batches each); the pairwise
tensors and the j-reduction stay local per core.

The problem is HBM-bound (ev alone is 12 MiB/core in f32), so everything
rides in bf16: ev/mask/s stream in as bf16 (mask is 0/1 so bf16 is exact),
v streams out as bf16 and is upcast on host. That halves HBM traffic vs the
f32 baseline. ev is pre-transposed on host to [j_local, c, (jc,it,i)] so the
contraction over j needs NO on-device transposes: the masked ev chunks feed
the PE directly as the moving operand with h[jc] stationary.

Per-core plan (B_loc=4, N=512, H=128, C=3), per batch:
  h-phase: rank-1 matmul seeds PSUM with b1 (ones^T @ b1rep), 4 bf16 matmuls
  accumulate s @ W1.T on top; LayerNorm stats via one bn_stats + 4 bn_aggr
  reading PSUM; ACT computes h = Silu(psum*rstd - mu*rstd) per tile straight
  from PSUM into bf16 SBUF (one fused op: scale/bias are per-partition APs).
  i-phase: one 1.5 MiB DMA brings evT[b], one 0.5 MiB DMA brings maskT[b];
  a single DVE multiply applies the mask (broadcast over c on the outer free
  dim, operands contiguous so the 16-bit 2x mode can engage); 16 bf16
  matmuls (h[jc] stationary, mev[c,jc,it] moving) accumulate v into 4 PSUM
  banks; ACT evicts to bf16 and the store rides the ACT HWDGE ring so it
  can't block the SP-ring loads. Host reorders [d,(it,c,i)] -> [i,c,d].
"""

import sys

if "/opt/trn_rl_repo" not in sys.path:
    sys.path.insert(0, "/opt/trn_rl_repo")

from contextlib import ExitStack

import numpy as np
import ml_dtypes

import concourse.bass as bass
import concourse.mybir as mybir
from concourse.tile import TileContext

B, N, H, C = 32, 512, 128, 3
NCORES = 8
BL = B // NCORES      # batches per core
P = 128
NT = N // P           # i-tiles per batch
JC = N // P           # j-chunks
LN_EPS = 1e-5
F32 = mybir.dt.float32
BF16 = mybir.dt.bfloat16
AF = mybir.ActivationFunctionType
BF16NP = ml_dtypes.bfloat16

JNP = JC * NT * P     # flattened (jc, it, i) extent = 2048


def _split_multi_waits(nc):
    """The walrus build in this container only accepts one sync-wait per
    instruction; hoist extra waits onto single-wait NOPs in front."""
    ctr = 0
    for f in nc.m.functions:
        for bb in f.blocks:
            insts = bb.instructions
            i = 0
            while i < len(insts):
                inst = insts[i]
                si = inst.sync_info
                if si is not None and len(si.on_wait) > 1:
                    waits = list(si.on_wait)
                    for w in waits[:-1]:
                        ctr += 1
                        nop = mybir.InstNoOp(
                            name=f"splitwait-{ctr}",
                            engine=inst.engine,
                            sync_info=mybir.SyncInfo(on_wait=[w], on_update=[]),
                            bass_nofuse=True,
                        )
                        nc.register_instruction(nop, overwrite=True)
                        insts.insert(i, nop)
                        i += 1
                    inst.sync_info = mybir.SyncInfo(
                        on_wait=[waits[-1]], on_update=list(si.on_update)
                    )
                i += 1


def build(reps=1):
    nc = bass.Bass("TRN2", target_bir_lowering=False, debug=False, num_devices=NCORES)
    evT = nc.dram_tensor("evT", [BL, P, C * JNP], BF16, kind="ExternalInput").ap()
    maskT = nc.dram_tensor("maskT", [BL, P, JNP], BF16, kind="ExternalInput").ap()
    sT = nc.dram_tensor("sT", [BL, H, N], BF16, kind="ExternalInput").ap()
    w1t = nc.dram_tensor("w1t", [H, H], BF16, kind="ExternalInput").ap()
    b1rep = nc.dram_tensor("b1rep", [1, NT * H], BF16, kind="ExternalInput").ap()
    out = nc.dram_tensor("out", [BL, H, NT * C * P], BF16, kind="ExternalOutput").ap()

    with TileContext(nc) as tc, ExitStack() as ctx:
        const = ctx.enter_context(tc.tile_pool(name="const", bufs=1))
        p_ev = ctx.enter_context(tc.tile_pool(name="p_ev", bufs=2))
        p_mask = ctx.enter_context(tc.tile_pool(name="p_mask", bufs=2))
        p_mev = ctx.enter_context(tc.tile_pool(name="p_mev", bufs=2))
        p_vout = ctx.enter_context(tc.tile_pool(name="p_vout", bufs=2))
        p_sT = ctx.enter_context(tc.tile_pool(name="p_sT", bufs=2))
        p_h = ctx.enter_context(tc.tile_pool(name="p_h", bufs=2))
        p_stat = ctx.enter_context(tc.tile_pool(name="p_stat", bufs=4))
        ps_h = ctx.enter_context(tc.tile_pool(name="ps_h", bufs=2, space="PSUM"))
        ps_v = ctx.enter_context(tc.tile_pool(name="ps_v", bufs=1, space="PSUM"))

        w1t_sb = const.tile([H, H], BF16)
        nc.sync.dma_start(out=w1t_sb[:], in_=w1t[:])
        b1rep_sb = const.tile([1, NT * H], BF16)
        nc.sync.dma_start(out=b1rep_sb[:], in_=b1rep[:])
        ones_sb = const.tile([1, P], BF16)
        nc.vector.memset(ones_sb[:], 1.0)
        eps_sb = const.tile([P, 1], F32)
        nc.vector.memset(eps_sb[:], LN_EPS)

        def body():
          for b in range(BL):
            # ---------- h phase: h = silu(LN(s @ W1.T + b1)) ----------
            sT_sb = p_sT.tile([H, N], BF16)
            nc.sync.dma_start(out=sT_sb[:], in_=sT[b])
            # big streaming loads issued right behind sT on the SP ring
            mk_sb = p_mask.tile([P, JNP], BF16)
            nc.sync.dma_start(out=mk_sb[:], in_=maskT[b])
            ev_sb = p_ev.tile([P, C, JNP], BF16)
            nc.sync.dma_start(out=ev_sb[:].rearrange("p c j -> p (c j)"), in_=evT[b])

            psum_h = ps_h.tile([P, NT * H], F32)
            # seed all of PSUM with b1 (rank-1: ones^T @ b1rep), then
            # accumulate the 4 n-tile matmuls on top
            nc.tensor.matmul(
                out=psum_h[:],
                lhsT=ones_sb[:],
                rhs=b1rep_sb[:],
                start=True,
                stop=False,
                skip_group_check=True,
            )
            for t in range(NT):
                # out[n_local, k] = sum_h sT[h, n] * W1T[h, k]
                nc.tensor.matmul(
                    out=psum_h[:, t * H : (t + 1) * H],
                    lhsT=sT_sb[:, t * P : (t + 1) * P],
                    rhs=w1t_sb[:],
                    start=False,
                    stop=True,
                    skip_group_check=True,
                )

            # LN stats straight off PSUM, then per-tile aggregation + fused
            # normalize+SiLU on ACT
            h_sb = p_h.tile([P, NT, H], BF16)
            for t in range(NT):
                stats = p_stat.tile([P, 6], F32, tag="stats")
                nc.vector.bn_stats(
                    out=stats[:], in_=psum_h[:, t * H : (t + 1) * H]
                )
                mv = p_stat.tile([P, 2], F32, tag="mv")
                nc.vector.bn_aggr(out=mv[:], in_=stats[:])
                rstd = p_stat.tile([P, 1], F32, tag="rstd")
                nc.scalar.activation(
                    out=rstd[:], in_=mv[:, 1:2], func=AF.Sqrt, bias=eps_sb[:]
                )
                nc.vector.reciprocal(out=rstd[:], in_=rstd[:])
                nmr = p_stat.tile([P, 1], F32, tag="nmr")
                nc.vector.tensor_scalar(
                    out=nmr[:],
                    in0=mv[:, 0:1],
                    scalar1=rstd[:],
                    scalar2=-1.0,
                    op0=mybir.AluOpType.mult,
                    op1=mybir.AluOpType.mult,
                )
                # h = Silu(x * rstd - mu * rstd) straight from PSUM -> bf16
                nc.scalar.activation(
                    out=h_sb[:, t, :],
                    in_=psum_h[:, t * H : (t + 1) * H],
                    func=AF.Silu,
                    bias=nmr[:],
                    scale=rstd[:],
                )

            # ---------- i phase ----------
            # mev[j, c, (jc,it,i)] = ev[j, c, (jc,it,i)] * mask[j, (jc,it,i)]
            mev_sb = p_mev.tile([P, C, JNP], BF16)
            nc.vector.tensor_tensor(
                out=mev_sb[:],
                in0=ev_sb[:],
                in1=mk_sb[:].unsqueeze(1).broadcast_to((P, C, JNP)),
                op=mybir.AluOpType.mult,
            )

            # v[d, (c,i)] += sum_j h[j, d] * mev[j, (c,i)] ; PSUM bank per it
            psum_v = ps_v.tile([P, NT, 512], F32)
            for jc in range(JC):
                for it in range(NT):
                    off = jc * NT * P + it * P
                    nc.tensor.matmul(
                        out=psum_v[:, it, : C * P],
                        lhsT=h_sb[:, jc, :],
                        rhs=mev_sb[:, :, off : off + P],
                        start=(jc == 0),
                        stop=(jc == JC - 1),
                        skip_group_check=True,
                    )

            vout = p_vout.tile([P, NT, C * P], BF16)
            nc.scalar.activation(
                out=vout[:], in_=psum_v[:, :, : C * P], func=AF.Copy
            )
            # store on the ACT HWDGE ring so a compute-gated store can't
            # block the next batch's loads on the SP HWDGE FIFO
            nc.scalar.dma_start(
                out=out[b], in_=vout[:].rearrange("p t x -> p (t x)")
            )

        if reps == 1:
            body()
        else:
            with tc.For_i(0, reps, 1):
                body()

    _split_multi_waits(nc)
    return nc


_built_nc = None


def _get_nc():
    global _built_nc
    if _built_nc is None:
        _built_nc = build()
    return _built_nc


def shard_inputs(s, ev, mask, W1, b1):
    """Full inputs -> list of per-core input dicts (bf16, pre-transposed)."""
    s = np.asarray(s, dtype=np.float32)
    ev = np.asarray(ev, dtype=np.float32)
    mask = np.asarray(mask, dtype=np.float32)
    W1 = np.asarray(W1, dtype=np.float32)
    b1 = np.asarray(b1, dtype=np.float32)
    w1t = np.ascontiguousarray(W1.T).astype(BF16NP)
    b1rep = np.tile(b1, NT)[None, :].astype(BF16NP)
    in_maps = []
    for m in range(NCORES):
        bs = slice(m * BL, (m + 1) * BL)
        # ev[b, i, j, c] -> evT[b, j_local, c, jc, it, i_local]
        evm = ev[bs].reshape(BL, NT, P, JC, P, C)
        evm = evm.transpose(0, 4, 5, 3, 1, 2).reshape(BL, P, C * JNP)
        # mask[b, i, j, 1] -> maskT[b, j_local, jc, it, i_local]
        mkm = mask[bs].reshape(BL, NT, P, JC, P)
        mkm = mkm.transpose(0, 4, 3, 1, 2).reshape(BL, P, JNP)
        in_maps.append(
            {
                "evT": np.ascontiguousarray(evm).astype(BF16NP),
                "maskT": np.ascontiguousarray(mkm).astype(BF16NP),
                "sT": np.ascontiguousarray(s[bs].transpose(0, 2, 1)).astype(BF16NP),
                "w1t": w1t,
                "b1rep": b1rep,
            }
        )
    return in_maps


def unshard_output(per_core_outs):
    """list of per-core "out" arrays [BL, H, NT*C*P] -> full (B, N, 3, H)."""
    parts = []
    for o in per_core_outs:
        o = np.asarray(o, dtype=np.float32).reshape(BL, H, NT, C, P)
        o = o.transpose(0, 2, 4, 3, 1)  # [BL, it, i, c, d]
        parts.append(np.ascontiguousarray(o).reshape(BL, N, C, H))
    return np.concatenate(parts, axis=0)


_executor = None


def _get_executor():
    """Build the sharded PJRT executable once; reuse across kernel() calls."""
    global _executor
    if _executor is not None:
        return _executor
    import jax
    from jax.sharding import Mesh, PartitionSpec
    from jax.experimental.shard_map import shard_map

    from concourse import bass2jax

    bass2jax.install_neuronx_cc_hook()
    nc = _get_nc()
    partition_name = nc.partition_id_tensor.name if nc.partition_id_tensor else None
    in_names, out_names, out_avals, zero_outs = [], [], [], []
    for alloc in nc.m.functions[0].allocations:
        if not isinstance(alloc, mybir.MemoryLocationSet):
            continue
        name = alloc.memorylocations[0].name
        if alloc.kind == "ExternalInput":
            if name != partition_name:
                in_names.append(name)
        elif alloc.kind == "ExternalOutput":
            out_names.append(name)
            shape = tuple(alloc.tensor_shape)
            dtype = mybir.dt.np(alloc.dtype)
            out_avals.append(jax.core.ShapedArray(shape, dtype))
            zero_outs.append(np.zeros(shape, dtype))
    n_params = len(in_names)
    all_in_names = list(in_names) + list(out_names)
    if partition_name is not None:
        all_in_names.append(partition_name)

    def _body(*args):
        operands = list(args)
        if partition_name is not None:
            operands.append(bass2jax.partition_id_tensor())
        outs = bass2jax._bass_exec_p.bind(
            *operands,
            out_avals=tuple(out_avals),
            in_names=tuple(all_in_names),
            out_names=tuple(out_names),
            lowering_input_output_aliases=(),
            sim_require_finite=True,
            sim_require_nnan=True,
            nc=nc,
        )
        return tuple(outs)

    devices = jax.devices()[:NCORES]
    mesh = Mesh(np.asarray(devices), ("core",))
    donate = tuple(range(n_params, n_params + len(out_names)))
    fn = jax.jit(
        shard_map(
            _body,
            mesh=mesh,
            in_specs=(PartitionSpec("core"),) * (n_params + len(out_names)),
            out_specs=(PartitionSpec("core"),) * len(out_names),
            check_rep=False,
        ),
        donate_argnums=donate,
        keep_unused=True,
    )
    _executor = (fn, in_names, out_names, out_avals, zero_outs)
    return _executor


def kernel(s, ev, mask, W1, b1):
    fn, in_names, out_names, out_avals, zero_outs = _get_executor()
    in_maps = shard_inputs(s, ev, mask, W1, b1)
    concat_in = [
        np.concatenate([in_maps[c][nm] for c in range(NCORES)], axis=0)
        for nm in in_names
    ]
    concat_zeros = [
        np.zeros((NCORES * z.shape[0], *z.shape[1:]), z.dtype) for z in zero_outs
    ]
    out_arrs = fn(*concat_in, *concat_zeros)
    i = out_names.index("out")
    o = np.asarray(out_arrs[i]).reshape(NCORES, *out_avals[i].shape)
    return unshard_output([o[c] for c in range(NCORES)])


# revision 5
# speedup vs baseline: 1.3619x; 1.2895x over previous
"""CFConvS2V Trainium2 kernel (8-core data-parallel over batch).

reference computation:
    h = silu(layernorm(s @ W1.T + b1))               # (B, N, H)
    v[b,i,c,d] = sum_j mask[b,i,j] * ev[b,i,j,c] * h[b,j,d]   # (B, N, 3, H)

Sharding: data-parallel over B across 8 cores (4 batches each); the pairwise
tensors and the j-reduction stay local per core.

The problem is HBM-bound (ev alone is 12 MiB/core in f32), so everything
rides in bf16: ev/mask/s stream in as bf16 (mask is 0/1 so bf16 is exact),
v streams out as bf16 and is upcast on host. That halves HBM traffic vs the
f32 baseline. ev is pre-transposed on host to [j_local, c, (jc,it,i)] so the
contraction over j needs NO on-device transposes: the masked ev chunks feed
the PE directly as the moving operand with h[jc] stationary.

Per-core plan (B_loc=4, N=512, H=128, C=3), per batch:
  h-phase: rank-1 matmul seeds PSUM with b1 (ones^T @ b1rep), 4 bf16 matmuls
  accumulate s @ W1.T on top; LayerNorm stats via one bn_stats + 4 bn_aggr
  reading PSUM; ACT computes h = Silu(psum*rstd - mu*rstd) per tile straight
  from PSUM into bf16 SBUF (one fused op: scale/bias are per-partition APs).
  i-phase: one 1.5 MiB DMA brings evT[b], one 0.5 MiB DMA brings maskT[b];
  a single DVE multiply applies the mask (broadcast over c on the outer free
  dim, operands contiguous so the 16-bit 2x mode can engage); 16 bf16
  matmuls (h[jc] stationary, mev[c,jc,it] moving) accumulate v into 4 PSUM
  banks; ACT evicts to bf16 and the store rides the ACT HWDGE ring so it
  can't block the SP-ring loads. Host reorders [d,(it,c,i)] -> [i,c,d].
"""

import sys

if "/opt/trn_rl_repo" not in sys.path:
    sys.path.insert(0, "/opt/trn_rl_repo")

from contextlib import ExitStack

import numpy as np
import ml_dtypes

import concourse.bass as bass
import concourse.mybir as mybir
from concourse.tile import TileContext

B, N, H, C = 32, 512, 128, 3
NCORES = 8
BL = B // NCORES      # batches per core
P = 128
NT = N // P           # i-tiles per batch
JC = N // P           # j-chunks
LN_EPS = 1e-5
F32 = mybir.dt.float32
BF16 = mybir.dt.bfloat16
AF = mybir.ActivationFunctionType
BF16NP = ml_dtypes.bfloat16

JNP = JC * NT * P     # flattened (jc, it, i) extent = 2048


def _split_multi_waits(nc):
    """The walrus build in this container only accepts one sync-wait per
    instruction; hoist extra waits onto single-wait NOPs in front."""
    ctr = 0
    for f in nc.m.functions:
        for bb in f.blocks:
            insts = bb.instructions
            i = 0
            while i < len(insts):
                inst = insts[i]
                si = inst.sync_info
                if si is not None and len(si.on_wait) > 1:
                    waits = list(si.on_wait)
                    for w in waits[:-1]:
                        ctr += 1
                        nop = mybir.InstNoOp(
                            name=f"splitwait-{ctr}",
                            engine=inst.engine,
                            sync_info=mybir.SyncInfo(on_wait=[w], on_update=[]),
                            bass_nofuse=True,
                        )
                        nc.register_instruction(nop, overwrite=True)
                        insts.insert(i, nop)
                        i += 1
                    inst.sync_info = mybir.SyncInfo(
                        on_wait=[waits[-1]], on_update=list(si.on_update)
                    )
                i += 1


def build(reps=1):
    nc = bass.Bass("TRN2", target_bir_lowering=False, debug=False, num_devices=NCORES)
    evT = nc.dram_tensor("evT", [BL, P, C * JNP], BF16, kind="ExternalInput").ap()
    maskT = nc.dram_tensor("maskT", [BL, P, JNP], BF16, kind="ExternalInput").ap()
    sT = nc.dram_tensor("sT", [BL, H, N], BF16, kind="ExternalInput").ap()
    w1t = nc.dram_tensor("w1t", [H, H], BF16, kind="ExternalInput").ap()
    b1rep = nc.dram_tensor("b1rep", [1, NT * H], BF16, kind="ExternalInput").ap()
    out = nc.dram_tensor("out", [BL, H, NT * C * P], BF16, kind="ExternalOutput").ap()

    with TileContext(nc) as tc, ExitStack() as ctx:
        const = ctx.enter_context(tc.tile_pool(name="const", bufs=1))
        p_ev = ctx.enter_context(tc.tile_pool(name="p_ev", bufs=2))
        p_mask = ctx.enter_context(tc.tile_pool(name="p_mask", bufs=2))
        p_mev = ctx.enter_context(tc.tile_pool(name="p_mev", bufs=2))
        p_vout = ctx.enter_context(tc.tile_pool(name="p_vout", bufs=2))
        p_sT = ctx.enter_context(tc.tile_pool(name="p_sT", bufs=2))
        p_h = ctx.enter_context(tc.tile_pool(name="p_h", bufs=2))
        p_stat = ctx.enter_context(tc.tile_pool(name="p_stat", bufs=4))
        ps_h = ctx.enter_context(tc.tile_pool(name="ps_h", bufs=2, space="PSUM"))
        ps_v = ctx.enter_context(tc.tile_pool(name="ps_v", bufs=1, space="PSUM"))

        w1t_sb = const.tile([H, H], BF16)
        nc.sync.dma_start(out=w1t_sb[:], in_=w1t[:])
        b1rep_sb = const.tile([1, NT * H], BF16)
        nc.sync.dma_start(out=b1rep_sb[:], in_=b1rep[:])
        ones_sb = const.tile([1, P], BF16)
        nc.vector.memset(ones_sb[:], 1.0)

        def body():
          for b in range(BL):
            # ---------- h phase: h = silu(LN(s @ W1.T + b1)) ----------
            sT_sb = p_sT.tile([H, N], BF16)
            nc.sync.dma_start(out=sT_sb[:], in_=sT[b])
            # big streaming loads issued right behind sT on the SP ring
            mk_sb = p_mask.tile([P, JNP], BF16)
            nc.sync.dma_start(out=mk_sb[:], in_=maskT[b])
            ev_sb = p_ev.tile([P, C, JNP], BF16)
            nc.sync.dma_start(out=ev_sb[:].rearrange("p c j -> p (c j)"), in_=evT[b])

            psum_h = ps_h.tile([P, NT * H], F32)
            # seed all of PSUM with b1 (rank-1: ones^T @ b1rep), then
            # accumulate the 4 n-tile matmuls on top
            nc.tensor.matmul(
                out=psum_h[:],
                lhsT=ones_sb[:],
                rhs=b1rep_sb[:],
                start=True,
                stop=False,
                skip_group_check=True,
            )
            for t in range(NT):
                # out[n_local, k] = sum_h sT[h, n] * W1T[h, k]
                nc.tensor.matmul(
                    out=psum_h[:, t * H : (t + 1) * H],
                    lhsT=sT_sb[:, t * P : (t + 1) * P],
                    rhs=w1t_sb[:],
                    start=False,
                    stop=True,
                    skip_group_check=True,
                )

            # LN stats straight off PSUM; rstd entirely on DVE (Newton with
            # fast-inverse-sqrt seed) so ACT only ever runs Silu/Copy — both
            # live in the silu_and_others table set, so NO ~1.3us
            # ACT_TABLE_LOADs in steady state (Sqrt lives in another set).
            mvall = p_stat.tile([P, NT, 2], F32, tag="mv")
            for t in range(NT):
                stats = p_stat.tile([P, 6], F32, tag="stats")
                nc.vector.bn_stats(
                    out=stats[:], in_=psum_h[:, t * H : (t + 1) * H]
                )
                nc.vector.bn_aggr(out=mvall[:, t, :], in_=stats[:])
            # v = var + eps   (batched over the 4 tiles: [p, NT])
            v4 = p_stat.tile([P, NT], F32, tag="v4")
            nc.vector.tensor_scalar(
                out=v4[:], in0=mvall[:, :, 1], scalar1=LN_EPS, scalar2=None,
                op0=mybir.AluOpType.add,
            )
            # y0 = bitcast(0x5f3759df - (bitcast(v) >> 1))
            yi4 = p_stat.tile([P, NT], mybir.dt.int32, tag="yi4")
            nc.vector.tensor_scalar(
                out=yi4[:], in0=v4[:].bitcast(mybir.dt.int32), scalar1=1,
                scalar2=None, op0=mybir.AluOpType.logical_shift_right,
            )
            nc.vector.tensor_scalar(
                out=yi4[:], in0=yi4[:], scalar1=-1, scalar2=0x5F3759DF,
                op0=mybir.AluOpType.mult, op1=mybir.AluOpType.add,
            )
            # two Newton steps: y = y*(1.5 - 0.5*v*y^2)  -> rstd to ~1e-5
            rstd4 = yi4[:].bitcast(F32)
            t14 = p_stat.tile([P, NT], F32, tag="t14")
            for _ in range(2):
                nc.vector.tensor_mul(out=t14[:], in0=rstd4, in1=rstd4)
                nc.vector.tensor_mul(out=t14[:], in0=t14[:], in1=v4[:])
                nc.vector.tensor_scalar(
                    out=t14[:], in0=t14[:], scalar1=-0.5, scalar2=1.5,
                    op0=mybir.AluOpType.mult, op1=mybir.AluOpType.add,
                )
                nc.vector.tensor_mul(out=yi4[:].bitcast(F32), in0=rstd4, in1=t14[:])
            # nmr = -mu * rstd
            nmr4 = p_stat.tile([P, NT], F32, tag="nmr4")
            nc.vector.tensor_mul(out=nmr4[:], in0=mvall[:, :, 0], in1=rstd4)
            nc.vector.tensor_scalar(
                out=nmr4[:], in0=nmr4[:], scalar1=-1.0, scalar2=None,
                op0=mybir.AluOpType.mult,
            )
            h_sb = p_h.tile([P, NT, H], BF16)
            for t in range(NT):
                # h = Silu(x * rstd - mu * rstd) straight from PSUM -> bf16
                nc.scalar.activation(
                    out=h_sb[:, t, :],
                    in_=psum_h[:, t * H : (t + 1) * H],
                    func=AF.Silu,
                    bias=nmr4[:, t : t + 1],
                    scale=rstd4[:, t : t + 1],
                )

            # ---------- i phase ----------
            # mev[j, c, (jc,it,i)] = ev[j, c, (jc,it,i)] * mask[j, (jc,it,i)]
            mev_sb = p_mev.tile([P, C, JNP], BF16)
            nc.vector.tensor_tensor(
                out=mev_sb[:],
                in0=ev_sb[:],
                in1=mk_sb[:].unsqueeze(1).broadcast_to((P, C, JNP)),
                op=mybir.AluOpType.mult,
            )

            # v[d, (c,i)] += sum_j h[j, d] * mev[j, (c,i)] ; PSUM bank per it
            psum_v = ps_v.tile([P, NT, 512], F32)
            for jc in range(JC):
                for it in range(NT):
                    off = jc * NT * P + it * P
                    nc.tensor.matmul(
                        out=psum_v[:, it, : C * P],
                        lhsT=h_sb[:, jc, :],
                        rhs=mev_sb[:, :, off : off + P],
                        start=(jc == 0),
                        stop=(jc == JC - 1),
                        skip_group_check=True,
                    )

            vout = p_vout.tile([P, NT, C * P], BF16)
            nc.scalar.activation(
                out=vout[:], in_=psum_v[:, :, : C * P], func=AF.Copy
            )
            # store on the ACT HWDGE ring so a compute-gated store can't
            # block the next batch's loads on the SP HWDGE FIFO
            nc.scalar.dma_start(
                out=out[b], in_=vout[:].rearrange("p t x -> p (t x)")
            )

        if reps == 1:
            body()
        else:
            with tc.For_i(0, reps, 1):
                body()

    _split_multi_waits(nc)
    return nc


_built_nc = None


def _get_nc():
    global _built_nc
    if _built_nc is None:
        _built_nc = build()
    return _built_nc


def shard_inputs(s, ev, mask, W1, b1):
    """Full inputs -> list of per-core input dicts (bf16, pre-transposed)."""
    s = np.asarray(s, dtype=np.float32)
    ev = np.asarray(ev, dtype=np.float32)
    mask = np.asarray(mask, dtype=np.float32)
    W1 = np.asarray(W1, dtype=np.float32)
    b1 = np.asarray(b1, dtype=np.float32)
    w1t = np.ascontiguousarray(W1.T).astype(BF16NP)
    b1rep = np.tile(b1, NT)[None, :].astype(BF16NP)
    in_maps = []
    for m in range(NCORES):
        bs = slice(m * BL, (m + 1) * BL)
        # ev[b, i, j, c] -> evT[b, j_local, c, jc, it, i_local]
        evm = ev[bs].reshape(BL, NT, P, JC, P, C)
        evm = evm.transpose(0, 4, 5, 3, 1, 2).reshape(BL, P, C * JNP)
        # mask[b, i, j, 1] -> maskT[b, j_local, jc, it, i_local]
        mkm = mask[bs].reshape(BL, NT, P, JC, P)
        mkm = mkm.transpose(0, 4, 3, 1, 2).reshape(BL, P, JNP)
        in_maps.append(
            {
                "evT": np.ascontiguousarray(evm).astype(BF16NP),
                "maskT": np.ascontiguousarray(mkm).astype(BF16NP),
                "sT": np.ascontiguousarray(s[bs].transpose(0, 2, 1)).astype(BF16NP),
                "w1t": w1t,
                "b1rep": b1rep,
            }
        )
    return in_maps


def unshard_output(per_core_outs):
    """list of per-core "out" arrays [BL, H, NT*C*P] -> full (B, N, 3, H)."""
    parts = []
    for o in per_core_outs:
        o = np.asarray(o, dtype=np.float32).reshape(BL, H, NT, C, P)
        o = o.transpose(0, 2, 4, 3, 1)  # [BL, it, i, c, d]
        parts.append(np.ascontiguousarray(o).reshape(BL, N, C, H))
    return np.concatenate(parts, axis=0)


_executor = None


def _get_executor():
    """Build the sharded PJRT executable once; reuse across kernel() calls."""
    global _executor
    if _executor is not None:
        return _executor
    import jax
    from jax.sharding import Mesh, PartitionSpec
    from jax.experimental.shard_map import shard_map

    from concourse import bass2jax

    bass2jax.install_neuronx_cc_hook()
    nc = _get_nc()
    partition_name = nc.partition_id_tensor.name if nc.partition_id_tensor else None
    in_names, out_names, out_avals, zero_outs = [], [], [], []
    for alloc in nc.m.functions[0].allocations:
        if not isinstance(alloc, mybir.MemoryLocationSet):
            continue
        name = alloc.memorylocations[0].name
        if alloc.kind == "ExternalInput":
            if name != partition_name:
                in_names.append(name)
        elif alloc.kind == "ExternalOutput":
            out_names.append(name)
            shape = tuple(alloc.tensor_shape)
            dtype = mybir.dt.np(alloc.dtype)
            out_avals.append(jax.core.ShapedArray(shape, dtype))
            zero_outs.append(np.zeros(shape, dtype))
    n_params = len(in_names)
    all_in_names = list(in_names) + list(out_names)
    if partition_name is not None:
        all_in_names.append(partition_name)

    def _body(*args):
        operands = list(args)
        if partition_name is not None:
            operands.append(bass2jax.partition_id_tensor())
        outs = bass2jax._bass_exec_p.bind(
            *operands,
            out_avals=tuple(out_avals),
            in_names=tuple(all_in_names),
            out_names=tuple(out_names),
            lowering_input_output_aliases=(),
            sim_require_finite=True,
            sim_require_nnan=True,
            nc=nc,
        )
        return tuple(outs)

    devices = jax.devices()[:NCORES]
    mesh = Mesh(np.asarray(devices), ("core",))
    donate = tuple(range(n_params, n_params + len(out_names)))
    fn = jax.jit(
        shard_map(
            _body,
            mesh=mesh,
            in_specs=(PartitionSpec("core"),) * (n_params + len(out_names)),
            out_specs=(PartitionSpec("core"),) * len(out_names),
            check_rep=False,
        ),
        donate_argnums=donate,
        keep_unused=True,
    )
    _executor = (fn, in_names, out_names, out_avals, zero_outs)
    return _executor


def kernel(s, ev, mask, W1, b1):
    fn, in_names, out_names, out_avals, zero_outs = _get_executor()
    in_maps = shard_inputs(s, ev, mask, W1, b1)
    concat_in = [
        np.concatenate([in_maps[c][nm] for c in range(NCORES)], axis=0)
        for nm in in_names
    ]
    concat_zeros = [
        np.zeros((NCORES * z.shape[0], *z.shape[1:]), z.dtype) for z in zero_outs
    ]
    out_arrs = fn(*concat_in, *concat_zeros)
    i = out_names.index("out")
    o = np.asarray(out_arrs[i]).reshape(NCORES, *out_avals[i].shape)
    return unshard_output([o[c] for c in range(NCORES)])
